# revision 19
# baseline (speedup 1.0000x reference)
"""MultiHeadLatentAttention TRN2 kernel — 8-core tensor-parallel over heads.

Strategy:
  - Host fuses latent down-projections into the up-projections (weight
    absorption): q = x @ (W_q_down @ [W_qc|W_qr]), k_c/v = x @ (W_kv_down @ .),
    so every projection is head-shardable. 2 heads per core; the score scale
    1/sqrt(HD) is folded into the q weights.
  - Host pre-transposes x -> xT [D, B*S] so all device matmuls read natural
    layouts. Device computes qT/kT per head ([head_dim, tok], RoPE applied at
    partitions 64:128 via a PE permutation matmul + DVE mul/add), v natural
    [tok, head_dim] via PE transpose of vT.
  - Causal attention per (batch, head, 128-row q-block), q-blocks processed
    in pairs so the AV matmul runs at N=256: scores on PE, softmax without
    max-subtraction (logits are bounded: |s| < ~4 so exp cannot overflow),
    exp with accumulated row-sums on ACT, normalization on DVE, attn
    transposed back via PE (pairs of 128-blocks per PSUM tile).
  - Per-core out-projection partial (ctx_local @ W_o rows); host sums the 8
    partials + b_o.
  - Stream dtype MLA_DT: bf16 (default, ~4e-3 rel err, PE full rate) |
    f32r (~2e-4 rel err but compiles to the 2-pass fp32 PE path, ~1.6x
    slower) | f32.
"""
import os
import numpy as np

import concourse.bass as bass
import concourse.mybir as mybir
from concourse.tile import TileContext
from concourse.bass_utils import run_bass_kernel_spmd

F32 = mybir.dt.float32
AF = mybir.ActivationFunctionType
ALU = mybir.AluOpType

NC = 8           # cores
HL = 2           # heads per core
B, S, D = 2, 2048, 2048
H, HD, RD, CD = 16, 128, 64, 64
TOK = B * S
TC = 512         # projection token chunk
NTC = S // TC    # 4 chunks per batch
ND = D // 128    # 16 contraction chunks
QB = 128         # q block
NQB = S // QB    # 16
KC = 512         # score k chunk

_DT_NAME = os.environ.get("MLA_DT", "bf16")     # bf16 | f32r | f32
_NORM_DIAG = os.environ.get("MLA_NORM", "pass") == "diag"

_CACHE = {}


# ---------------------------------------------------------------------------
# Tile-on-this-walrus compat: max ONE sync wait per instruction. Extra waits
# are hoisted onto wait-only EventSemaphore instructions inserted just before
# the over-subscribed instruction on the same engine (program order makes
# this equivalent).
# ---------------------------------------------------------------------------
def _split_multi_waits(nc, max_waits=1):
    n = 0
    for f in nc.m.functions:
        for bb in f.blocks:
            new_insts = []
            for ins in bb.instructions:
                si = ins.sync_info
                waits = list(si.on_wait) if si is not None else []
                if len(waits) > max_waits:
                    extra, keep = waits[:-max_waits], waits[-max_waits:]
                    for j, w in enumerate(extra):
                        ev = mybir.InstEventSemaphore(
                            name=f"{ins.name}_xw{j}",
                            engine=ins.engine,
                            ins=[],
                            outs=[],
                            sync_info=mybir.SyncInfo(on_wait=[w], on_update=[]),
                        )
                        new_insts.append(ev)
                        n += 1
                    ins.sync_info = mybir.SyncInfo(
                        on_wait=keep, on_update=list(si.on_update)
                    )
                new_insts.append(ins)
            bb.instructions[:] = new_insts
    return n


def _stream_dt():
    if _DT_NAME == "bf16":
        return mybir.dt.bfloat16
    if _DT_NAME == "f32":
        return mybir.dt.float32
    return mybir.dt.float32r


# ---------------------------------------------------------------------------
# Device program (SPMD: identical on all 8 cores, weights differ per core)
# ---------------------------------------------------------------------------
def _build_program():
    DT = _stream_dt()
    nc = bass.Bass()

    xT = nc.dram_tensor("xT", [D, TOK], DT, kind="ExternalInput")
    wq = nc.dram_tensor("wq", [D, HL * HD], DT, kind="ExternalInput")
    wk = nc.dram_tensor("wk", [D, HL * HD], DT, kind="ExternalInput")
    wv = nc.dram_tensor("wv", [D, HL * HD], DT, kind="ExternalInput")
    wo = nc.dram_tensor("wo", [HL * HD, D], DT, kind="ExternalInput")
    cosd = nc.dram_tensor("cosd", [RD, S], F32, kind="ExternalInput")
    sind = nc.dram_tensor("sind", [RD, S], F32, kind="ExternalInput")
    maskd = nc.dram_tensor("maskd", [QB, 4 * KC], F32, kind="ExternalInput")
    identd = nc.dram_tensor("identd", [128, 128], DT, kind="ExternalInput")
    p64d = nc.dram_tensor("p64d", [RD, RD], DT, kind="ExternalInput")
    outd = nc.dram_tensor("out", [TOK, D], F32, kind="ExternalOutput")

    with TileContext(nc) as tc:
        with tc.tile_pool(name="static", bufs=1) as stat, \
             tc.tile_pool(name="perb", bufs=2) as perb, \
             tc.tile_pool(name="attnp", bufs=4) as attnp, \
             tc.tile_pool(name="xtc", bufs=20) as xtc, \
             tc.tile_pool(name="stream", bufs=3) as stream, \
             tc.tile_pool(name="small", bufs=2) as small, \
             tc.tile_pool(name="psA", bufs=2, space="PSUM") as psA, \
             tc.tile_pool(name="psB", bufs=2, space="PSUM") as psB, \
             tc.tile_pool(name="psC", bufs=2, space="PSUM") as psC, \
             tc.tile_pool(name="psD", bufs=2, space="PSUM") as psD:

            # ---- stage constants/weights ----
            wq_sb = stat.tile([128, ND * HL * HD], DT, tag="wq")
            wk_sb = stat.tile([128, ND * HL * HD], DT, tag="wk")
            wv_sb = stat.tile([128, ND * HL * HD], DT, tag="wv")
            for csb, cdr in ((wq_sb, wq), (wk_sb, wk), (wv_sb, wv)):
                for d in range(ND):
                    nc.sync.dma_start(
                        out=csb[:, d * 256:(d + 1) * 256],
                        in_=cdr[d * 128:(d + 1) * 128, :])
            wo_sb = stat.tile([128, 2 * D], DT, tag="wo")
            for c in range(2):
                nc.sync.dma_start(out=wo_sb[:, c * D:(c + 1) * D],
                                  in_=wo[c * 128:(c + 1) * 128, :])
            cosT = stat.tile([128, S], F32, tag="cos")
            sinT = stat.tile([128, S], F32, tag="sin")
            nc.sync.dma_start(out=cosT[64:128, :], in_=cosd[:])
            nc.sync.dma_start(out=sinT[64:128, :], in_=sind[:])
            mask4 = stat.tile([QB, 4 * KC], F32, tag="mask")
            nc.sync.dma_start(out=mask4[:], in_=maskd[:])
            ident = stat.tile([128, 128], DT, tag="ident")
            nc.sync.dma_start(out=ident[:], in_=identd[:])
            p64 = stat.tile([128, RD], DT, tag="p64")
            nc.sync.dma_start(out=p64[64:128, :], in_=p64d[:])

            for b in range(B):
                g0 = b * S
                qT = [perb.tile([128, S], DT, tag=f"qT{l}", name=f"qT{l}") for l in range(HL)]
                kT = [perb.tile([128, S], DT, tag=f"kT{l}", name=f"kT{l}") for l in range(HL)]
                vN = [perb.tile([128, S], DT, tag=f"vN{l}", name=f"vN{l}") for l in range(HL)]

                # ---------------- projections ----------------
                # Stage the 16 xT tiles of each 512-token chunk in SBUF once,
                # then run both heads' q/k/vT accumulations from SBUF.
                for t in range(NTC):
                    sl = slice(t * TC, (t + 1) * TC)
                    xts = []
                    for d in range(ND):
                        xt = xtc.tile([128, TC], DT, tag="xt", name=f"xt{d}")
                        nc.sync.dma_start(
                            out=xt[:],
                            in_=xT[d * 128:(d + 1) * 128,
                                   g0 + t * TC: g0 + (t + 1) * TC])
                        xts.append(xt)
                    for l in range(HL):
                        q_ps = psA.tile([128, TC], F32, tag="mm512")
                        k_ps = psA.tile([128, TC], F32, tag="mm512")
                        for d in range(ND):
                            c0 = d * 256 + l * 128
                            st, sp = d == 0, d == ND - 1
                            nc.tensor.matmul(q_ps[:], wq_sb[:, c0:c0 + 128],
                                             xts[d][:], start=st, stop=sp)
                            nc.tensor.matmul(k_ps[:], wk_sb[:, c0:c0 + 128],
                                             xts[d][:], start=st, stop=sp)
                        v_ps = psA.tile([128, TC], F32, tag="mm512")
                        for d in range(ND):
                            c0 = d * 256 + l * 128
                            st, sp = d == 0, d == ND - 1
                            nc.tensor.matmul(v_ps[:], wv_sb[:, c0:c0 + 128],
                                             xts[d][:], start=st, stop=sp)
                        # q/k: copy + rope (rope rows at partitions 64:128)
                        for dst, src in ((qT[l], q_ps), (kT[l], k_ps)):
                            nc.vector.tensor_copy(dst[:, sl], src[:])
                            rot = psB.tile([128, TC], F32, tag="scrot")
                            nc.tensor.matmul(rot[0:64, :], p64[64:128, :],
                                             dst[64:128, sl],
                                             start=True, stop=True)
                            nc.gpsimd.tensor_tensor(
                                dst[64:128, sl], dst[64:128, sl],
                                cosT[64:128, sl], op=ALU.mult)
                            tmp = small.tile([128, TC], F32, tag="ropetmp")
                            nc.vector.tensor_tensor(
                                tmp[64:128, :], rot[0:64, :],
                                sinT[64:128, sl], op=ALU.mult)
                            nc.gpsimd.tensor_tensor(
                                dst[64:128, sl], dst[64:128, sl],
                                tmp[64:128, :], op=ALU.add)
                        # v: vT chunk -> natural layout via PE transpose
                        vt_sb = small.tile([128, TC], DT, tag="vtsb")
                        nc.vector.tensor_copy(vt_sb[:], v_ps[:])
                        for s4 in range(0, TC // 128, 2):
                            tp = psC.tile([128, 512], DT, tag="tp")
                            nc.tensor.transpose(
                                tp[:, 0:128],
                                vt_sb[:, s4 * 128:(s4 + 1) * 128], ident[:])
                            nc.tensor.transpose(
                                tp[:, 128:256],
                                vt_sb[:, (s4 + 1) * 128:(s4 + 2) * 128],
                                ident[:])
                            col = (t * (TC // 128) + s4) * 128
                            nc.scalar.copy(vN[l][:, col:col + 256],
                                           tp[:, 0:256])

                # -------- attention + out-projection (paired q-blocks) ------
                for pr in range(NQB // 2):
                    qb0, qb1 = 2 * pr, 2 * pr + 1
                    nk = qb1 // 4 + 1          # 512-wide k chunks
                    nkb = qb1 + 1              # 128-wide k blocks
                    ctxT = [None, None]
                    for l in range(HL):
                        attn = [None, None]
                        dg = [None, None]
                        for i, qb in enumerate((qb0, qb1)):
                            at = attnp.tile([128, S], DT, tag="attn")
                            racc = small.tile([128, 1], F32, tag="racc")
                            for kc in range(nk):
                                sc = psB.tile([128, KC], F32, tag="scrot")
                                nc.tensor.matmul(
                                    sc[:], qT[l][:, qb * 128:(qb + 1) * 128],
                                    kT[l][:, kc * KC:(kc + 1) * KC],
                                    start=True, stop=True)
                                acc = small.tile([128, 1], F32, tag="acc")
                                if kc == qb // 4:
                                    scm = small.tile([128, KC], F32, tag="scm")
                                    v4 = qb % 4
                                    nc.vector.tensor_tensor(
                                        scm[:], sc[:],
                                        mask4[:, v4 * KC:(v4 + 1) * KC],
                                        op=ALU.add)
                                    nc.scalar.activation(
                                        at[:, kc * KC:(kc + 1) * KC], scm[:],
                                        AF.Exp, accum_out=acc[:])
                                else:
                                    nc.scalar.activation(
                                        at[:, kc * KC:(kc + 1) * KC], sc[:],
                                        AF.Exp, accum_out=acc[:])
                                if kc == 0:
                                    nc.gpsimd.tensor_copy(racc[:], acc[:])
                                else:
                                    nc.gpsimd.tensor_tensor(
                                        racc[:], racc[:], acc[:], op=ALU.add)
                            rc = small.tile([128, 1], F32, tag="recip")
                            nc.vector.reciprocal(rc[:], racc[:])
                            attn[i] = at
                            if _NORM_DIAG:
                                dgt = small.tile([128, 128], DT, tag="diag")
                                nc.vector.tensor_scalar_mul(
                                    dgt[:], ident[:], rc[:])
                                dg[i] = dgt
                            else:
                                nc.vector.tensor_scalar_mul(
                                    at[:, 0:nk * KC], at[:, 0:nk * KC], rc[:])
                                dg[i] = ident
                        # ctx: transpose attn blocks (2 k-blocks x 2 q-blocks
                        # per PSUM tile), accumulate AV at N=256
                        ctx_ps = psD.tile([128, 256], F32, tag="ctx")
                        for kb in range(0, nkb, 2):
                            atT = small.tile([128, 512], DT, tag="atT")
                            tp = psC.tile([128, 512], DT, tag="tp")
                            for j in range(2):
                                for i in range(2):
                                    nc.tensor.transpose(
                                        tp[:, j * 256 + i * 128:
                                           j * 256 + (i + 1) * 128],
                                        attn[i][:, (kb + j) * 128:
                                                (kb + j + 1) * 128],
                                        dg[i][:])
                            if (kb // 2) % 2 == 0:
                                nc.scalar.copy(atT[:], tp[:])
                            else:
                                nc.vector.tensor_copy(atT[:], tp[:])
                            for j in range(2):
                                nc.tensor.matmul(
                                    ctx_ps[:],
                                    vN[l][:, (kb + j) * 128:(kb + j + 1) * 128],
                                    atT[:, j * 256:(j + 1) * 256],
                                    start=(kb + j == 0),
                                    stop=(kb + j == nkb - 1))
                        ct = small.tile([128, 256], DT, tag=f"ctxT{l}")
                        nc.vector.tensor_copy(ct[:], ctx_ps[:])
                        ctxT[l] = ct
                    # out-projection for this pair of q-blocks
                    for qh in range(2):
                        row0 = g0 + (2 * pr + qh) * 128
                        for n in range(4):
                            op_ps = psA.tile([128, 512], F32, tag="mm512")
                            nc.tensor.matmul(
                                op_ps[:], ctxT[0][:, qh * 128:(qh + 1) * 128],
                                wo_sb[:, n * 512:n * 512 + 512],
                                start=True, stop=False)
                            nc.tensor.matmul(
                                op_ps[:], ctxT[1][:, qh * 128:(qh + 1) * 128],
                                wo_sb[:, D + n * 512:D + n * 512 + 512],
                                start=False, stop=True)
                            ob = stream.tile([128, 512], F32, tag="outsb")
                            if n % 2 == 0:
                                nc.vector.tensor_copy(ob[:], op_ps[:])
                            else:
                                nc.scalar.copy(ob[:], op_ps[:])
                            nc.sync.dma_start(
                                out=outd[row0:row0 + 128,
                                         n * 512:(n + 1) * 512],
                                in_=ob[:])
    return nc


# ---------------------------------------------------------------------------
# Host side
# ---------------------------------------------------------------------------
def _rope_tables():
    inv_freq = 1.0 / (10000.0 ** (np.arange(0, RD, 2, dtype=np.float32) / RD))
    t = np.arange(S, dtype=np.float32)
    freqs = np.outer(t, inv_freq).astype(np.float32)
    emb = np.concatenate([freqs, freqs], axis=-1)
    cos = np.cos(emb).astype(np.float32)    # [S, RD]
    sin = np.sin(emb).astype(np.float32)
    return np.ascontiguousarray(cos.T), np.ascontiguousarray(sin.T)


def _host_prep(x, W_kv_down, W_q_down, W_kc, W_v, W_qc, W_kr, W_qr, W_o, b_o):
    f = np.float32
    Wqc_f = (W_q_down @ W_qc).astype(f)       # [D, CD*H]
    Wqr_f = (W_q_down @ W_qr).astype(f)       # [D, RD*H]
    Wkc_f = (W_kv_down @ W_kc).astype(f)      # [D, CD*H]
    Wv_f = (W_kv_down @ W_v).astype(f)        # [D, HD*H]
    scale = f(1.0 / np.sqrt(np.float32(HD)))

    xT = np.ascontiguousarray(x.reshape(TOK, D).T).astype(f)
    cosT, sinT = _rope_tables()

    mask4 = np.zeros((QB, 4 * KC), f)
    ii = np.arange(QB)[:, None]
    jj = np.arange(KC)[None, :]
    for v in range(4):
        mask4[:, v * KC:(v + 1) * KC] = np.where(jj <= v * 128 + ii,
                                                 0.0, -1e30)

    ident = np.eye(128, dtype=f)
    p64 = np.zeros((RD, RD), f)
    for m in range(RD):
        if m < 32:
            p64[m + 32, m] = -1.0
        else:
            p64[m - 32, m] = 1.0

    in_maps = []
    for c in range(NC):
        wq_c = np.empty((D, HL * HD), f)
        wk_c = np.empty((D, HL * HD), f)
        wv_c = np.empty((D, HL * HD), f)
        wo_c = np.empty((HL * HD, D), f)
        for l in range(HL):
            g = HL * c + l
            wq_c[:, l * 128:l * 128 + 64] = \
                Wqc_f[:, g * 64:(g + 1) * 64] * scale
            wq_c[:, l * 128 + 64:(l + 1) * 128] = \
                Wqr_f[:, g * 64:(g + 1) * 64] * scale
            wk_c[:, l * 128:l * 128 + 64] = Wkc_f[:, g * 64:(g + 1) * 64]
            wk_c[:, l * 128 + 64:(l + 1) * 128] = W_kr
            wv_c[:, l * 128:(l + 1) * 128] = Wv_f[:, g * 128:(g + 1) * 128]
            wo_c[l * 128:(l + 1) * 128, :] = W_o[g * 128:(g + 1) * 128, :]
        in_maps.append({
            "xT": xT, "wq": wq_c, "wk": wk_c, "wv": wv_c, "wo": wo_c,
            "cosd": cosT, "sind": sinT, "maskd": mask4,
            "identd": ident, "p64d": p64,
        })
    if _DT_NAME == "bf16":
        import ml_dtypes
        bf = ml_dtypes.bfloat16
        cast_keys = ("xT", "wq", "wk", "wv", "wo", "identd", "p64d")
        in_maps = [{k: (v.astype(bf) if k in cast_keys else v)
                    for k, v in m.items()} for m in in_maps]
    return in_maps


def kernel(**inputs):
    inputs = {k: np.asarray(v, np.float32) for k, v in inputs.items()}
    if "nc" not in _CACHE:
        prog = _build_program()
        _split_multi_waits(prog)
        _CACHE["nc"] = prog
    prog = _CACHE["nc"]
    in_maps = _host_prep(**inputs)
    res = None
    for attempt in range(3):
        try:
            res = run_bass_kernel_spmd(prog, in_maps, core_ids=list(range(NC)))
            break
        except Exception:
            if attempt == 2:
                raise
            import time
            time.sleep(5.0)
    out = np.zeros((TOK, D), np.float64)
    for r in res.results:
        out += r["out"]
    out = out.astype(np.float32) + inputs["b_o"][None, :]
    return out.reshape(B, S, D)


# revision 20
# speedup vs baseline: 1.2229x; 1.2229x over previous
"""MultiHeadLatentAttention TRN2 kernel — 8-core tensor-parallel over heads.

Strategy:
  - Host fuses latent down-projections into the up-projections (weight
    absorption): q = x @ (W_q_down @ [W_qc|W_qr]), k_c/v = x @ (W_kv_down @ .),
    so every projection is head-shardable. 2 heads per core; the score scale
    1/sqrt(HD) is folded into the q weights.
  - Host pre-transposes x -> xT [D, B*S] so all device matmuls read natural
    layouts. Device computes qT/kT per head ([head_dim, tok], RoPE applied at
    partitions 64:128 via a PE permutation matmul + DVE mul/add), v natural
    [tok, head_dim] via PE transpose of vT.
  - Causal attention per (batch, head, 128-row q-block), q-blocks processed
    in pairs so the AV matmul runs at N=256: scores on PE, softmax without
    max-subtraction (logits are bounded: |s| < ~4 so exp cannot overflow),
    exp with accumulated row-sums on ACT, normalization on DVE, attn
    transposed back via PE (pairs of 128-blocks per PSUM tile).
  - Per-core out-projection partial (ctx_local @ W_o rows); host sums the 8
    partials + b_o.
  - Stream dtype MLA_DT: bf16 (default, ~4e-3 rel err, PE full rate) |
    f32r (~2e-4 rel err but compiles to the 2-pass fp32 PE path, ~1.6x
    slower) | f32.
"""
import os
import numpy as np

import concourse.bass as bass
import concourse.mybir as mybir
from concourse.tile import TileContext
from concourse.bass_utils import run_bass_kernel_spmd

F32 = mybir.dt.float32
AF = mybir.ActivationFunctionType
ALU = mybir.AluOpType

NC = 8           # cores
HL = 2           # heads per core
B, S, D = 2, 2048, 2048
H, HD, RD, CD = 16, 128, 64, 64
TOK = B * S
TC = 512         # projection token chunk
NTC = S // TC    # 4 chunks per batch
ND = D // 128    # 16 contraction chunks
QB = 128         # q block
NQB = S // QB    # 16
KC = 512         # score k chunk

_DT_NAME = os.environ.get("MLA_DT", "bf16")     # bf16 | f32r | f32
_NORM_DIAG = os.environ.get("MLA_NORM", "pass") == "diag"

_CACHE = {}


# ---------------------------------------------------------------------------
# Tile-on-this-walrus compat: max ONE sync wait per instruction. Extra waits
# are hoisted onto wait-only EventSemaphore instructions inserted just before
# the over-subscribed instruction on the same engine (program order makes
# this equivalent).
# ---------------------------------------------------------------------------
def _split_multi_waits(nc, max_waits=1):
    n = 0
    for f in nc.m.functions:
        for bb in f.blocks:
            new_insts = []
            for ins in bb.instructions:
                si = ins.sync_info
                waits = list(si.on_wait) if si is not None else []
                if len(waits) > max_waits:
                    extra, keep = waits[:-max_waits], waits[-max_waits:]
                    for j, w in enumerate(extra):
                        ev = mybir.InstEventSemaphore(
                            name=f"{ins.name}_xw{j}",
                            engine=ins.engine,
                            ins=[],
                            outs=[],
                            sync_info=mybir.SyncInfo(on_wait=[w], on_update=[]),
                        )
                        new_insts.append(ev)
                        n += 1
                    ins.sync_info = mybir.SyncInfo(
                        on_wait=keep, on_update=list(si.on_update)
                    )
                new_insts.append(ins)
            bb.instructions[:] = new_insts
    return n


def _stream_dt():
    if _DT_NAME == "bf16":
        return mybir.dt.bfloat16
    if _DT_NAME == "f32":
        return mybir.dt.float32
    return mybir.dt.float32r


# ---------------------------------------------------------------------------
# Device program (SPMD: identical on all 8 cores, weights differ per core)
# ---------------------------------------------------------------------------
def _build_program():
    DT = _stream_dt()
    nc = bass.Bass()

    xT = nc.dram_tensor("xT", [D, TOK], DT, kind="ExternalInput")
    wq = nc.dram_tensor("wq", [D, HL * HD], DT, kind="ExternalInput")
    wk = nc.dram_tensor("wk", [D, HL * HD], DT, kind="ExternalInput")
    wv = nc.dram_tensor("wv", [D, HL * HD], DT, kind="ExternalInput")
    wo = nc.dram_tensor("wo", [HL * HD, D], DT, kind="ExternalInput")
    cosd = nc.dram_tensor("cosd", [RD, S], F32, kind="ExternalInput")
    sind = nc.dram_tensor("sind", [RD, S], F32, kind="ExternalInput")
    maskd = nc.dram_tensor("maskd", [QB, 4 * KC], F32, kind="ExternalInput")
    identd = nc.dram_tensor("identd", [128, 128], DT, kind="ExternalInput")
    p64d = nc.dram_tensor("p64d", [RD, RD], DT, kind="ExternalInput")
    outd = nc.dram_tensor("out", [TOK, D], F32, kind="ExternalOutput")

    with TileContext(nc) as tc:
        with tc.tile_pool(name="static", bufs=1) as stat, \
             tc.tile_pool(name="perb", bufs=2) as perb, \
             tc.tile_pool(name="attnp", bufs=6) as attnp, \
             tc.tile_pool(name="xtc", bufs=20) as xtc, \
             tc.tile_pool(name="stream", bufs=4) as stream, \
             tc.tile_pool(name="small", bufs=3) as small, \
             tc.tile_pool(name="psA", bufs=3, space="PSUM") as psA, \
             tc.tile_pool(name="psB", bufs=2, space="PSUM") as psB, \
             tc.tile_pool(name="psC", bufs=2, space="PSUM") as psC, \
             tc.tile_pool(name="psD", bufs=1, space="PSUM") as psD:

            # ---- stage constants/weights ----
            wq_sb = stat.tile([128, ND * HL * HD], DT, tag="wq")
            wk_sb = stat.tile([128, ND * HL * HD], DT, tag="wk")
            wv_sb = stat.tile([128, ND * HL * HD], DT, tag="wv")
            for csb, cdr in ((wq_sb, wq), (wk_sb, wk), (wv_sb, wv)):
                for d in range(ND):
                    nc.sync.dma_start(
                        out=csb[:, d * 256:(d + 1) * 256],
                        in_=cdr[d * 128:(d + 1) * 128, :])
            wo_sb = stat.tile([128, 2 * D], DT, tag="wo")
            for c in range(2):
                nc.sync.dma_start(out=wo_sb[:, c * D:(c + 1) * D],
                                  in_=wo[c * 128:(c + 1) * 128, :])
            cosT = stat.tile([128, S], F32, tag="cos")
            sinT = stat.tile([128, S], F32, tag="sin")
            nc.sync.dma_start(out=cosT[64:128, :], in_=cosd[:])
            nc.sync.dma_start(out=sinT[64:128, :], in_=sind[:])
            mask4 = stat.tile([QB, 4 * KC], F32, tag="mask")
            nc.sync.dma_start(out=mask4[:], in_=maskd[:])
            ident = stat.tile([128, 128], DT, tag="ident")
            nc.sync.dma_start(out=ident[:], in_=identd[:])
            p64 = stat.tile([128, RD], DT, tag="p64")
            nc.sync.dma_start(out=p64[64:128, :], in_=p64d[:])

            for b in range(B):
                g0 = b * S
                qT = [perb.tile([128, S], DT, tag=f"qT{l}", name=f"qT{l}") for l in range(HL)]
                kT = [perb.tile([128, S], DT, tag=f"kT{l}", name=f"kT{l}") for l in range(HL)]
                vN = [perb.tile([128, S], DT, tag=f"vN{l}", name=f"vN{l}") for l in range(HL)]

                # ---------------- projections ----------------
                # Stage the 16 xT tiles of each 512-token chunk in SBUF once,
                # then run both heads' q/k/vT accumulations from SBUF.
                for t in range(NTC):
                    sl = slice(t * TC, (t + 1) * TC)
                    xts = []
                    for d in range(ND):
                        xt = xtc.tile([128, TC], DT, tag="xt", name=f"xt{d}")
                        nc.sync.dma_start(
                            out=xt[:],
                            in_=xT[d * 128:(d + 1) * 128,
                                   g0 + t * TC: g0 + (t + 1) * TC])
                        xts.append(xt)
                    for l in range(HL):
                        q_ps = psA.tile([128, TC], F32, tag="mm512")
                        k_ps = psA.tile([128, TC], F32, tag="mm512")
                        v_ps = psA.tile([128, TC], F32, tag="mm512")
                        for d in range(ND):
                            c0 = d * 256 + l * 128
                            st, sp = d == 0, d == ND - 1
                            nc.tensor.matmul(q_ps[:], wq_sb[:, c0:c0 + 128],
                                             xts[d][:], start=st, stop=sp)
                            nc.tensor.matmul(k_ps[:], wk_sb[:, c0:c0 + 128],
                                             xts[d][:], start=st, stop=sp)
                            nc.tensor.matmul(v_ps[:], wv_sb[:, c0:c0 + 128],
                                             xts[d][:], start=st, stop=sp)
                        # q/k: copy + rope (rope rows at partitions 64:128)
                        for dst, src in ((qT[l], q_ps), (kT[l], k_ps)):
                            nc.vector.tensor_copy(dst[:, sl], src[:])
                            rot = psB.tile([128, TC], F32, tag="scrot")
                            nc.tensor.matmul(rot[0:64, :], p64[64:128, :],
                                             dst[64:128, sl],
                                             start=True, stop=True)
                            nc.vector.tensor_tensor(
                                dst[64:128, sl], dst[64:128, sl],
                                cosT[64:128, sl], op=ALU.mult)
                            tmp = small.tile([128, TC], F32, tag="ropetmp")
                            nc.vector.tensor_tensor(
                                tmp[64:128, :], rot[0:64, :],
                                sinT[64:128, sl], op=ALU.mult)
                            nc.vector.tensor_tensor(
                                dst[64:128, sl], dst[64:128, sl],
                                tmp[64:128, :], op=ALU.add)
                        # v: vT chunk -> natural layout via PE transpose
                        vt_sb = small.tile([128, TC], DT, tag="vtsb")
                        nc.vector.tensor_copy(vt_sb[:], v_ps[:])
                        for s4 in range(0, TC // 128, 2):
                            tp = psC.tile([128, 512], DT, tag="tp")
                            nc.tensor.transpose(
                                tp[:, 0:128],
                                vt_sb[:, s4 * 128:(s4 + 1) * 128], ident[:])
                            nc.tensor.transpose(
                                tp[:, 128:256],
                                vt_sb[:, (s4 + 1) * 128:(s4 + 2) * 128],
                                ident[:])
                            col = (t * (TC // 128) + s4) * 128
                            nc.scalar.copy(vN[l][:, col:col + 256],
                                           tp[:, 0:256])

                # -------- attention + out-projection (paired q-blocks) ------
                for pr in range(NQB // 2):
                    qb0, qb1 = 2 * pr, 2 * pr + 1
                    nk = qb1 // 4 + 1          # 512-wide k chunks
                    nkb = qb1 + 1              # 128-wide k blocks
                    ctxT = [None, None]
                    for l in range(HL):
                        attn = [None, None]
                        dg = [None, None]
                        for i, qb in enumerate((qb0, qb1)):
                            at = attnp.tile([128, S], DT, tag="attn")
                            racc = small.tile([128, 1], F32, tag="racc")
                            for kc in range(nk):
                                sc = psB.tile([128, KC], F32, tag="scrot")
                                nc.tensor.matmul(
                                    sc[:], qT[l][:, qb * 128:(qb + 1) * 128],
                                    kT[l][:, kc * KC:(kc + 1) * KC],
                                    start=True, stop=True)
                                acc = small.tile([128, 1], F32, tag="acc")
                                if kc == qb // 4:
                                    scm = small.tile([128, KC], F32, tag="scm")
                                    v4 = qb % 4
                                    nc.vector.tensor_tensor(
                                        scm[:], sc[:],
                                        mask4[:, v4 * KC:(v4 + 1) * KC],
                                        op=ALU.add)
                                    nc.scalar.activation(
                                        at[:, kc * KC:(kc + 1) * KC], scm[:],
                                        AF.Exp, accum_out=acc[:])
                                else:
                                    nc.scalar.activation(
                                        at[:, kc * KC:(kc + 1) * KC], sc[:],
                                        AF.Exp, accum_out=acc[:])
                                if kc == 0:
                                    nc.gpsimd.tensor_copy(racc[:], acc[:])
                                else:
                                    nc.gpsimd.tensor_tensor(
                                        racc[:], racc[:], acc[:], op=ALU.add)
                            rc = small.tile([128, 1], F32, tag="recip")
                            nc.vector.reciprocal(rc[:], racc[:])
                            attn[i] = at
                            if _NORM_DIAG:
                                dgt = small.tile([128, 128], DT, tag="diag")
                                nc.vector.tensor_scalar_mul(
                                    dgt[:], ident[:], rc[:])
                                dg[i] = dgt
                            else:
                                nc.vector.tensor_scalar_mul(
                                    at[:, 0:nk * KC], at[:, 0:nk * KC], rc[:])
                                dg[i] = ident
                        # ctx: transpose attn blocks (2 k-blocks x 2 q-blocks
                        # per PSUM tile), accumulate AV at N=256
                        ctx_ps = psD.tile([128, 256], F32, tag="ctx")
                        for kb in range(0, nkb, 2):
                            atT = small.tile([128, 512], DT, tag="atT")
                            tp = psC.tile([128, 512], DT, tag="tp")
                            for j in range(2):
                                for i in range(2):
                                    nc.tensor.transpose(
                                        tp[:, j * 256 + i * 128:
                                           j * 256 + (i + 1) * 128],
                                        attn[i][:, (kb + j) * 128:
                                                (kb + j + 1) * 128],
                                        dg[i][:])
                            if (kb // 2) % 2 == 0:
                                nc.scalar.copy(atT[:], tp[:])
                            else:
                                nc.vector.tensor_copy(atT[:], tp[:])
                            for j in range(2):
                                nc.tensor.matmul(
                                    ctx_ps[:],
                                    vN[l][:, (kb + j) * 128:(kb + j + 1) * 128],
                                    atT[:, j * 256:(j + 1) * 256],
                                    start=(kb + j == 0),
                                    stop=(kb + j == nkb - 1))
                        ct = small.tile([128, 256], DT, tag=f"ctxT{l}")
                        nc.vector.tensor_copy(ct[:], ctx_ps[:])
                        ctxT[l] = ct
                    # out-projection for this pair of q-blocks
                    for qh in range(2):
                        row0 = g0 + (2 * pr + qh) * 128
                        for n in range(4):
                            op_ps = psA.tile([128, 512], F32, tag="mm512")
                            nc.tensor.matmul(
                                op_ps[:], ctxT[0][:, qh * 128:(qh + 1) * 128],
                                wo_sb[:, n * 512:n * 512 + 512],
                                start=True, stop=False)
                            nc.tensor.matmul(
                                op_ps[:], ctxT[1][:, qh * 128:(qh + 1) * 128],
                                wo_sb[:, D + n * 512:D + n * 512 + 512],
                                start=False, stop=True)
                            ob = stream.tile([128, 512], F32, tag="outsb")
                            if n % 2 == 0:
                                nc.vector.tensor_copy(ob[:], op_ps[:])
                            else:
                                nc.scalar.copy(ob[:], op_ps[:])
                            nc.sync.dma_start(
                                out=outd[row0:row0 + 128,
                                         n * 512:(n + 1) * 512],
                                in_=ob[:])
    return nc


# ---------------------------------------------------------------------------
# Host side
# ---------------------------------------------------------------------------
def _rope_tables():
    inv_freq = 1.0 / (10000.0 ** (np.arange(0, RD, 2, dtype=np.float32) / RD))
    t = np.arange(S, dtype=np.float32)
    freqs = np.outer(t, inv_freq).astype(np.float32)
    emb = np.concatenate([freqs, freqs], axis=-1)
    cos = np.cos(emb).astype(np.float32)    # [S, RD]
    sin = np.sin(emb).astype(np.float32)
    return np.ascontiguousarray(cos.T), np.ascontiguousarray(sin.T)


def _host_prep(x, W_kv_down, W_q_down, W_kc, W_v, W_qc, W_kr, W_qr, W_o, b_o):
    f = np.float32
    Wqc_f = (W_q_down @ W_qc).astype(f)       # [D, CD*H]
    Wqr_f = (W_q_down @ W_qr).astype(f)       # [D, RD*H]
    Wkc_f = (W_kv_down @ W_kc).astype(f)      # [D, CD*H]
    Wv_f = (W_kv_down @ W_v).astype(f)        # [D, HD*H]
    scale = f(1.0 / np.sqrt(np.float32(HD)))

    xT = np.ascontiguousarray(x.reshape(TOK, D).T).astype(f)
    cosT, sinT = _rope_tables()

    mask4 = np.zeros((QB, 4 * KC), f)
    ii = np.arange(QB)[:, None]
    jj = np.arange(KC)[None, :]
    for v in range(4):
        mask4[:, v * KC:(v + 1) * KC] = np.where(jj <= v * 128 + ii,
                                                 0.0, -1e30)

    ident = np.eye(128, dtype=f)
    p64 = np.zeros((RD, RD), f)
    for m in range(RD):
        if m < 32:
            p64[m + 32, m] = -1.0
        else:
            p64[m - 32, m] = 1.0

    in_maps = []
    for c in range(NC):
        wq_c = np.empty((D, HL * HD), f)
        wk_c = np.empty((D, HL * HD), f)
        wv_c = np.empty((D, HL * HD), f)
        wo_c = np.empty((HL * HD, D), f)
        for l in range(HL):
            g = HL * c + l
            wq_c[:, l * 128:l * 128 + 64] = \
                Wqc_f[:, g * 64:(g + 1) * 64] * scale
            wq_c[:, l * 128 + 64:(l + 1) * 128] = \
                Wqr_f[:, g * 64:(g + 1) * 64] * scale
            wk_c[:, l * 128:l * 128 + 64] = Wkc_f[:, g * 64:(g + 1) * 64]
            wk_c[:, l * 128 + 64:(l + 1) * 128] = W_kr
            wv_c[:, l * 128:(l + 1) * 128] = Wv_f[:, g * 128:(g + 1) * 128]
            wo_c[l * 128:(l + 1) * 128, :] = W_o[g * 128:(g + 1) * 128, :]
        in_maps.append({
            "xT": xT, "wq": wq_c, "wk": wk_c, "wv": wv_c, "wo": wo_c,
            "cosd": cosT, "sind": sinT, "maskd": mask4,
            "identd": ident, "p64d": p64,
        })
    if _DT_NAME == "bf16":
        import ml_dtypes
        bf = ml_dtypes.bfloat16
        cast_keys = ("xT", "wq", "wk", "wv", "wo", "identd", "p64d")
        in_maps = [{k: (v.astype(bf) if k in cast_keys else v)
                    for k, v in m.items()} for m in in_maps]
    return in_maps


def kernel(**inputs):
    inputs = {k: np.asarray(v, np.float32) for k, v in inputs.items()}
    if "nc" not in _CACHE:
        prog = _build_program()
        _split_multi_waits(prog)
        _CACHE["nc"] = prog
    prog = _CACHE["nc"]
    in_maps = _host_prep(**inputs)
    res = None
    for attempt in range(3):
        try:
            res = run_bass_kernel_spmd(prog, in_maps, core_ids=list(range(NC)))
            break
        except Exception:
            if attempt == 2:
                raise
            import time
            time.sleep(5.0)
    out = np.zeros((TOK, D), np.float64)
    for r in res.results:
        out += r["out"]
    out = out.astype(np.float32) + inputs["b_o"][None, :]
    return out.reshape(B, S, D)
